# revision 37
# baseline (speedup 1.0000x reference)
"""AdaWinBlock1d Trainium2 kernel — 8 NeuronCores, data-parallel over batch.

Reference computation (per batch, C=512, T=2048, S=128, W=37):
    mask = t < length                       [T]
    A    = mask / (win_sum(mask) + eps)     [T]       (host, from lengths only)
    ws   = win_sum(s)                       [S, T]    (device, shift-add tree)
    g    = fc_w @ (ws * A)                  [2C, T]   (win_sum linearity + column scaling)
    xn   = tanh(alpha * x)
    y    = (1 + g_gamma) * xn + g_beta
    h    = leaky_relu(y, 0.2)
    c    = conv1d(h, w, b)   (kernel 3, pad 1; 3 shifted matmuls in PSUM)
    ... second adawin + conv ...
    out  = (c2 + x) / sqrt(2)               (1/sqrt2 folded into conv2 weights & pre-scaled x)
"""

import sys, types, os

sys.path.insert(0, '/opt/trn_rl_repo')

# ---------------------------------------------------------------------------
# Shim antenv.axon_hooks (missing in this image) so trace=True works.
# ---------------------------------------------------------------------------
if "antenv.axon_hooks" not in sys.modules:
    _m = types.ModuleType("antenv.axon_hooks")
    _m._hook = None
    def _set_hook(h):
        _m._hook = h
    def _get_hook():
        return _m._hook
    _m.set_axon_ntff_profile_hook = _set_hook
    _m.get_axon_ntff_profile_hook = _get_hook
    sys.modules["antenv.axon_hooks"] = _m
    try:
        import antenv
        antenv.axon_hooks = _m
        from trn_agent_boot.trn_boot import _ntff_profile_via_ctypes
        hook = _ntff_profile_via_ctypes('/opt/axon/libaxon_pjrt.so')
        if hook is not None:
            _set_hook(hook)
    except Exception:
        pass

import numpy as np
import ml_dtypes

import concourse.bass_utils as bass_utils
bass_utils.upload_artifacts = lambda tmpdir: tmpdir  # no cloud store here

import concourse.bass as bass
import concourse.tile as tile
from concourse import mybir, bacc
from concourse.bass_utils import run_bass_kernel_spmd

F32 = mybir.dt.float32
BF16 = mybir.dt.bfloat16
AF = mybir.ActivationFunctionType

# Problem constants (hardcoded per spec)
B, C, T, S = 16, 512, 2048, 128
NCORES = 8
BL = B // NCORES          # batches per core
W_LEN = 37
PAD = W_LEN // 2          # 18
EPS = 1e-9
SLOPE = 0.2
SQRT2 = 1.4142135623730951
G = C // 128              # 4 channel groups
NT = T // 512             # 4 time chunks
P0W = T + 2 * PAD         # 2084


def _build(nc, fc1b_nz, fc2b_nz, cb2_nz, alpha1, alpha2):
    """Build the per-core Tile program. Shapes are per-core (BL batches)."""
    xs_d = nc.dram_tensor("xs", [BL, G, 128, T], BF16, kind="ExternalInput").ap()
    s_d = nc.dram_tensor("s", [BL, 128, T], F32, kind="ExternalInput").ap()
    a_d = nc.dram_tensor("a", [BL, T], BF16, kind="ExternalInput").ap()
    fc1w_d = nc.dram_tensor("fc1w", [128, 2 * C], BF16, kind="ExternalInput").ap()
    fc2w_d = nc.dram_tensor("fc2w", [128, 2 * C], BF16, kind="ExternalInput").ap()
    c1w_d = nc.dram_tensor("c1w", [128, 3, G, C], BF16, kind="ExternalInput").ap()
    c2w_d = nc.dram_tensor("c2w", [128, 3, G, C], BF16, kind="ExternalInput").ap()
    ab2_d = nc.dram_tensor("ab2", [G, 128], F32, kind="ExternalInput").ap()   # alpha2*conv1_b tiled
    cb2_d = nc.dram_tensor("cb2", [G, 128], F32, kind="ExternalInput").ap()   # conv2_b/sqrt2 tiled
    # cnt*A rows for the (normally absent) fc-bias path
    ca_d = nc.dram_tensor("ca", [BL, T], BF16, kind="ExternalInput").ap()
    fb1_d = nc.dram_tensor("fb1", [1, 2 * C], BF16, kind="ExternalInput").ap()
    fb2_d = nc.dram_tensor("fb2", [1, 2 * C], BF16, kind="ExternalInput").ap()
    out_d = nc.dram_tensor("out", [BL, G, 128, T], F32, kind="ExternalOutput").ap()

    dma = nc.sync.dma_start

    with tile.TileContext(nc) as tc:
        with (
            tc.tile_pool(name="wpool", bufs=1) as wpool,
            tc.tile_pool(name="batch", bufs=2) as pb,
            tc.tile_pool(name="tree", bufs=2) as tr,
            tc.tile_pool(name="small", bufs=6) as sm,
            tc.tile_pool(name="pg", bufs=4, space="PSUM") as pg,
            tc.tile_pool(name="pc", bufs=4, space="PSUM") as pc,
        ):
            # ---- DMA priority order: batch-0 style inputs first (they gate
            # the windowed-sum tree, which gates everything), then fc1 weights,
            # then batch-0 x, then conv weights, then batch-1 inputs. ----
            abc0 = pb.tile([128, T], BF16, name="abc0", tag="abc")
            dma(out=abc0[:, :], in_=bass.AP(tensor=a_d.tensor, offset=a_d.offset,
                                            ap=[[0, 128], [1, T]]))
            p00 = tr.tile([128, P0W], F32, name="p00", tag="p0")
            nc.vector.memset(p00[:, 0:PAD], 0.0)
            nc.vector.memset(p00[:, T + PAD:P0W], 0.0)
            # split so the first tree chunk's slice lands first
            for (c0, c1) in ((0, 530), (530, 1042), (1042, 1554), (1554, 2048)):
                dma(out=p00[:, PAD + c0:PAD + c1], in_=s_d[0, :, c0:c1])

            # PE warm-up: dummy matmuls bridge the tree latency and flip the
            # HAM clock gate to 2.4 GHz before the real matmul stream starts.
            def warmup(tag, cnt, rhs=None, lhsT=None, uid=[0]):
                for _ in range(cnt):
                    uid[0] += 1
                    pwu = pc.tile([128, 512], F32, name=f"pwu{uid[0]}", tag=tag)
                    nc.tensor.matmul(pwu[:, :],
                                     lhsT if lhsT is not None else abc0[:, 0:128],
                                     rhs if rhs is not None else abc0[:, 0:512],
                                     start=True, stop=True)

            warmup("pc", 14)

            fc1w = wpool.tile([128, 2 * C], BF16, name="fc1w")
            dma(out=fc1w[:, :], in_=fc1w_d[:, :])
            xs0 = pb.tile([128, G, T], BF16, name="xs0", tag="xs")
            # chunks 0..1 of every channel group land first (they gate epi0/epi1)
            for g_ in range(G):
                dma(out=xs0[:, g_, 0:1024], in_=xs_d[0, g_, :, 0:1024])
            for g_ in range(G):
                dma(out=xs0[:, g_, 1024:T], in_=xs_d[0, g_, :, 1024:T])
            c1w = wpool.tile([128, 3, G, C], BF16, name="c1w")
            dma(out=c1w[:, :, :, :], in_=c1w_d[:, :, :, :])
            fc2w = wpool.tile([128, 2 * C], BF16, name="fc2w")
            dma(out=fc2w[:, :], in_=fc2w_d[:, :])
            c2w = wpool.tile([128, 3, G, C], BF16, name="c2w")
            dma(out=c2w[:, :, :, :], in_=c2w_d[:, :, :, :])
            ab2 = wpool.tile([128, G], F32, name="ab2")
            dma(out=ab2[:, :], in_=bass.AP(tensor=ab2_d.tensor, offset=ab2_d.offset,
                                           ap=[[1, 128], [128, G]]))
            cb2 = wpool.tile([128, G], F32, name="cb2")
            dma(out=cb2[:, :], in_=bass.AP(tensor=cb2_d.tensor, offset=cb2_d.offset,
                                           ap=[[1, 128], [128, G]]))
            if fc1b_nz or fc2b_nz:
                fb1 = wpool.tile([1, 2 * C], BF16, name="fb1")
                dma(out=fb1[:, :], in_=fb1_d[:, :])
                fb2 = wpool.tile([1, 2 * C], BF16, name="fb2")
                dma(out=fb2[:, :], in_=fb2_d[:, :])

            for b in range(BL):
                # ---- load batch inputs ----
                if b == 0:
                    abc, p0 = abc0, p00
                else:
                    abc = pb.tile([128, T], BF16, name=f"abc{b}", tag="abc")
                    dma(out=abc[:, :], in_=bass.AP(tensor=a_d.tensor,
                                                   offset=a_d.offset + b * T,
                                                   ap=[[0, 128], [1, T]]))
                    p0 = tr.tile([128, P0W], F32, name=f"p0{b}", tag="p0")
                    nc.vector.memset(p0[:, 0:PAD], 0.0)
                    nc.vector.memset(p0[:, T + PAD:P0W], 0.0)
                    dma(out=p0[:, PAD:T + PAD], in_=s_d[b, :, :])
                if b == 0:
                    xs = xs0
                else:
                    xs = pb.tile([128, G, T], BF16, name=f"xs{b}", tag="xs")
                    dma(out=xs[:, :, :], in_=xs_d[b, :, :, :].rearrange("g p t -> p g t"))
                if fc1b_nz or fc2b_nz:
                    ca = pb.tile([1, T], BF16, name=f"ca{b}", tag="ca")
                    dma(out=ca[:, :], in_=ca_d[b:b + 1, :])

                # ---- windowed-sum tree chunk: p0 cols [o, o+CW) -> wsa[:, o:o+512) ----
                wsa = pb.tile([128, T], BF16, name=f"wsa{b}", tag="wsa")
                CW = 512 + 36  # chunk input width in p0 coords

                def tree_chunk(n):
                    o = n * 512
                    pin = p0[:, o:o + CW]
                    t2 = tr.tile([128, CW - 1], BF16, name=f"t2_{b}_{n}", tag="t2")
                    nc.vector.tensor_add(t2[:, :], pin[:, 0:CW - 1], pin[:, 1:CW])
                    t4 = tr.tile([128, CW - 3], BF16, name=f"t4_{b}_{n}", tag="t4")
                    nc.vector.tensor_add(t4[:, :], t2[:, 0:CW - 3], t2[:, 2:CW - 1])
                    t8 = tr.tile([128, CW - 7], BF16, name=f"t8_{b}_{n}", tag="t8")
                    nc.vector.tensor_add(t8[:, :], t4[:, 0:CW - 7], t4[:, 4:CW - 3])
                    t16 = tr.tile([128, CW - 15], BF16, name=f"t16_{b}_{n}", tag="t16")
                    nc.vector.tensor_add(t16[:, :], t8[:, 0:CW - 15], t8[:, 8:CW - 7])
                    t32 = tr.tile([128, CW - 31], BF16, name=f"t32_{b}_{n}", tag="t32")
                    nc.vector.tensor_add(t32[:, :], t16[:, 0:CW - 31], t16[:, 16:CW - 15])
                    t36 = tr.tile([128, CW - 35], BF16, name=f"t36_{b}_{n}", tag="t36")
                    nc.vector.tensor_add(t36[:, :], t32[:, 0:CW - 35], t4[:, 32:CW - 3])
                    ws = tr.tile([128, 512], BF16, name=f"ws{b}_{n}", tag="ws")
                    nc.vector.tensor_add(ws[:, :], t36[:, 0:512], pin[:, 36:CW])
                    nc.vector.tensor_mul(wsa[:, o:o + 512], ws[:, :], abc[:, o:o + 512])

                for _n in range(NT):
                    tree_chunk(_n)

                # ---- stage 1: g1 matmuls + epilogue -> h1 ----
                h1 = pb.tile([128, G, T + 4], BF16, name=f"h1_{b}", tag="h1")
                h2 = pb.tile([128, G, T + 4], BF16, name=f"h2_{b}", tag="h2", bufs=1)
                for m in range(G):
                    nc.vector.memset(h1[:, m, 0:2], 0.0)
                    nc.vector.memset(h1[:, m, T + 2:T + 4], 0.0)
                    nc.vector.memset(h2[:, m, 0:2], 0.0)
                    nc.vector.memset(h2[:, m, T + 2:T + 4], 0.0)

                def adawin_tile(m, n, fcw, fbt, fb_nz, xn_tile, h_out, uid,
                                fast_ramp=False):
                    """gamma/beta matmuls + (1+g)*xn + b epilogue + lrelu -> h_out chunk."""
                    ns = slice(n * 512, (n + 1) * 512)
                    pga = pg.tile([128, 512], F32, name=f"pga{uid}", tag="pg")
                    pgb = pg.tile([128, 512], F32, name=f"pgb{uid}", tag="pg")
                    if fb_nz:
                        nc.tensor.matmul(pga[:, :], fbt[:, m * 128:(m + 1) * 128],
                                         ca[:, ns], start=True, stop=False)
                        nc.tensor.matmul(pga[:, :], fcw[:, m * 128:(m + 1) * 128],
                                         wsa[:, ns], start=False, stop=True)
                        nc.tensor.matmul(pgb[:, :], fbt[:, C + m * 128:C + (m + 1) * 128],
                                         ca[:, ns], start=True, stop=False)
                        nc.tensor.matmul(pgb[:, :], fcw[:, C + m * 128:C + (m + 1) * 128],
                                         wsa[:, ns], start=False, stop=True)
                    else:
                        nc.tensor.matmul(pga[:, :], fcw[:, m * 128:(m + 1) * 128],
                                         wsa[:, ns], start=True, stop=True)
                        nc.tensor.matmul(pgb[:, :], fcw[:, C + m * 128:C + (m + 1) * 128],
                                         wsa[:, ns], start=True, stop=True)
                    if fast_ramp:
                        # ACT evacuates the gamma bank and folds the +1; keeps
                        # the ramp's serial DVE chain short (ACT is idle here).
                        cg = sm.tile([128, 512], BF16, name=f"cg{uid}", tag="u")
                        nc.scalar.activation(cg[:, :], pga[:, :], AF.Identity,
                                             bias=1.0)
                        u = sm.tile([128, 512], BF16, name=f"uf{uid}", tag="v")
                        nc.vector.tensor_mul(u[:, :], cg[:, :], xn_tile[:, :])
                        w = sm.tile([128, 512], BF16, name=f"w{uid}", tag="w")
                        nc.vector.tensor_add(w[:, :], u[:, :], pgb[:, :])
                    else:
                        u = sm.tile([128, 512], BF16, name=f"u{uid}", tag="u")
                        nc.vector.tensor_mul(u[:, :], pga[:, :], xn_tile[:, :])
                        v = sm.tile([128, 512], BF16, name=f"v{uid}", tag="v")
                        nc.vector.tensor_add(v[:, :], u[:, :], xn_tile[:, :])
                        w = sm.tile([128, 512], BF16, name=f"w{uid}", tag="w")
                        nc.vector.tensor_add(w[:, :], v[:, :], pgb[:, :])
                    nc.scalar.activation(h_out[:, m, 2 + n * 512:2 + (n + 1) * 512],
                                         w[:, :], AF.Prelu, alpha=SLOPE)

                for n in range(NT):
                    for m in range(G):
                        xn = sm.tile([128, 512], BF16, name=f"xn1_{b}_{m}_{n}", tag="xn1")
                        nc.scalar.activation(xn, xs[:, m, n * 512:(n + 1) * 512],
                                             AF.Tanh, scale=alpha1 * SQRT2)
                        adawin_tile(m, n, fc1w, fb1 if fc1b_nz else None,
                                    fc1b_nz, xn, h1, f"1_{b}_{m}_{n}",
                                    fast_ramp=(n < 2))
                    if b == 0 and n < 2:
                        warmup("pc", 10, rhs=wsa[:, n * 512:(n + 1) * 512])

                # ---- conv1 -> tanh -> stage 2 -> h2 ----
                for n in range(NT):
                    for m in range(G):
                        pct = pc.tile([128, 512], F32, name=f"pc1_{b}_{m}_{n}", tag="pc")
                        for k in range(3):
                            for ki in range(G):
                                lhsT = c1w[:, k, ki, m * 128:(m + 1) * 128]
                                rhs = h1[:, ki, 1 + k + n * 512:1 + k + (n + 1) * 512]
                                nc.tensor.matmul(pct[:, :], lhsT, rhs,
                                                 start=(k == 0 and ki == 0),
                                                 stop=(k == 2 and ki == G - 1))
                        xn2 = sm.tile([128, 512], BF16, name=f"xn2_{b}_{m}_{n}", tag="xn2")
                        nc.scalar.activation(xn2, pct[:, :], AF.Tanh,
                                             bias=ab2[:, m:m + 1], scale=alpha2)
                        adawin_tile(m, n, fc2w, None if not fc2b_nz else fb2,
                                    fc2b_nz, xn2, h2, f"2_{b}_{m}_{n}")

                # ---- conv2 + residual -> out ----
                for n in range(NT):
                    for m in range(G):
                        pct = pc.tile([128, 512], F32, name=f"pc2_{b}_{m}_{n}", tag="pc")
                        for k in range(3):
                            for ki in range(G):
                                lhsT = c2w[:, k, ki, m * 128:(m + 1) * 128]
                                rhs = h2[:, ki, 1 + k + n * 512:1 + k + (n + 1) * 512]
                                nc.tensor.matmul(pct[:, :], lhsT, rhs,
                                                 start=(k == 0 and ki == 0),
                                                 stop=(k == 2 and ki == G - 1))
                        ot = sm.tile([128, 512], F32, name=f"ot{b}_{m}_{n}", tag="ot")
                        nc.vector.tensor_add(ot[:, :], pct[:, :],
                                             xs[:, m, n * 512:(n + 1) * 512])
                        if cb2_nz:
                            nc.scalar.activation(ot[:, :], ot[:, :], AF.Identity,
                                                 bias=cb2[:, m:m + 1])
                        dma(out=out_d[b, m, :, n * 512:(n + 1) * 512], in_=ot[:, :])
    return nc


_CACHE = {}


def _get_nc(fc1b_nz, fc2b_nz, cb2_nz, alpha1, alpha2):
    key = (fc1b_nz, fc2b_nz, cb2_nz, float(alpha1), float(alpha2))
    if key not in _CACHE:
        nc = bacc.Bacc("TRN2", target_bir_lowering=False, debug=False,
                       num_devices=NCORES)
        _build(nc, fc1b_nz, fc2b_nz, cb2_nz, alpha1, alpha2)
        nc.compile()
        _CACHE[key] = nc
    return _CACHE[key]


def _host_prep(x, s, lengths, fc1_w, fc1_b, alpha1, conv1_w, conv1_b,
               fc2_w, fc2_b, alpha2, conv2_w, conv2_b):
    """Host-side input preparation. Returns (in_maps, meta)."""
    x = np.asarray(x, np.float32)
    s = np.asarray(s, np.float32)
    lengths = np.asarray(lengths)
    a1 = float(np.asarray(alpha1).reshape(-1)[0])
    a2 = float(np.asarray(alpha2).reshape(-1)[0])

    # A = mask / (win_sum(mask) + eps), cnt = win_sum(ones)  -- all [B, T]
    t_idx = np.arange(T)
    mask = (t_idx[None, :] < lengths[:, None]).astype(np.float64)
    kern = np.ones(W_LEN)
    den = np.stack([np.convolve(mask[i], kern, mode="same") for i in range(B)]) + EPS
    A = (mask / den).astype(ml_dtypes.bfloat16)
    cnt = np.convolve(np.ones(T), kern, mode="same")
    cA = (A * cnt[None, :]).astype(ml_dtypes.bfloat16)

    bf = ml_dtypes.bfloat16
    xs = (x / SQRT2).reshape(B, G, 128, T).astype(bf)
    fc1wT = np.ascontiguousarray(fc1_w.T).astype(bf)             # [S, 2C]
    fc2wT = np.ascontiguousarray(fc2_w.T).astype(bf)
    # conv weights: [O, I, 3] -> [p=i%128, k, ki=i//128, o]
    c1wT = np.ascontiguousarray(
        conv1_w.astype(np.float32).transpose(1, 2, 0).reshape(G, 128, 3, C)
        .transpose(1, 2, 0, 3)).astype(bf)
    c2wT = np.ascontiguousarray(
        (conv2_w.astype(np.float32) / SQRT2).transpose(1, 2, 0).reshape(G, 128, 3, C)
        .transpose(1, 2, 0, 3)).astype(bf)
    ab2 = (a2 * conv1_b.astype(np.float32)).reshape(G, 128)
    cb2 = (conv2_b.astype(np.float32) / SQRT2).reshape(G, 128)
    fb1 = fc1_b.astype(bf).reshape(1, 2 * C)
    fb2 = fc2_b.astype(bf).reshape(1, 2 * C)
    fc1b_nz = bool(np.any(fc1_b))
    fc2b_nz = bool(np.any(fc2_b))
    cb2_nz = bool(np.any(conv2_b))

    in_maps = []
    for c in range(NCORES):
        bs = slice(c * BL, (c + 1) * BL)
        in_maps.append({
            "xs": xs[bs], "s": s[bs], "a": A[bs], "ca": cA[bs],
            "fc1w": fc1wT, "fc2w": fc2wT, "c1w": c1wT, "c2w": c2wT,
            "ab2": ab2, "cb2": cb2, "fb1": fb1, "fb2": fb2,
        })
    return in_maps, (fc1b_nz, fc2b_nz, cb2_nz, a1, a2)


def kernel(x, s, lengths, fc1_w, fc1_b, alpha1, conv1_w, conv1_b,
           fc2_w, fc2_b, alpha2, conv2_w, conv2_b, _trace=False):
    in_maps, (fc1b_nz, fc2b_nz, cb2_nz, a1, a2) = _host_prep(
        x, s, lengths, fc1_w, fc1_b, alpha1, conv1_w, conv1_b,
        fc2_w, fc2_b, alpha2, conv2_w, conv2_b)
    nc = _get_nc(fc1b_nz, fc2b_nz, cb2_nz, a1, a2)
    res = run_bass_kernel_spmd(nc, in_maps, core_ids=list(range(NCORES)),
                               trace=_trace)
    out = np.concatenate([res.results[i]["out"] for i in range(NCORES)], axis=0)
    out = out.reshape(B, C, T).astype(np.float32)
    kernel.last_exec_time_ns = res.exec_time_ns
    return out


# revision 38
# speedup vs baseline: 1.0079x; 1.0079x over previous
"""AdaWinBlock1d Trainium2 kernel — 8 NeuronCores, data-parallel over batch.

Reference computation (per batch, C=512, T=2048, S=128, W=37):
    mask = t < length                       [T]
    A    = mask / (win_sum(mask) + eps)     [T]       (host, from lengths only)
    ws   = win_sum(s)                       [S, T]    (device, shift-add tree)
    g    = fc_w @ (ws * A)                  [2C, T]   (win_sum linearity + column scaling)
    xn   = tanh(alpha * x)
    y    = (1 + g_gamma) * xn + g_beta
    h    = leaky_relu(y, 0.2)
    c    = conv1d(h, w, b)   (kernel 3, pad 1; 3 shifted matmuls in PSUM)
    ... second adawin + conv ...
    out  = (c2 + x) / sqrt(2)               (1/sqrt2 folded into conv2 weights & pre-scaled x)
"""

import sys, types, os

sys.path.insert(0, '/opt/trn_rl_repo')

# ---------------------------------------------------------------------------
# Shim antenv.axon_hooks (missing in this image) so trace=True works.
# ---------------------------------------------------------------------------
if "antenv.axon_hooks" not in sys.modules:
    _m = types.ModuleType("antenv.axon_hooks")
    _m._hook = None
    def _set_hook(h):
        _m._hook = h
    def _get_hook():
        return _m._hook
    _m.set_axon_ntff_profile_hook = _set_hook
    _m.get_axon_ntff_profile_hook = _get_hook
    sys.modules["antenv.axon_hooks"] = _m
    try:
        import antenv
        antenv.axon_hooks = _m
        from trn_agent_boot.trn_boot import _ntff_profile_via_ctypes
        hook = _ntff_profile_via_ctypes('/opt/axon/libaxon_pjrt.so')
        if hook is not None:
            _set_hook(hook)
    except Exception:
        pass

import numpy as np
import ml_dtypes

import concourse.bass_utils as bass_utils
bass_utils.upload_artifacts = lambda tmpdir: tmpdir  # no cloud store here

import concourse.bass as bass
import concourse.tile as tile
from concourse import mybir, bacc
from concourse.bass_utils import run_bass_kernel_spmd

F32 = mybir.dt.float32
BF16 = mybir.dt.bfloat16
AF = mybir.ActivationFunctionType

# Problem constants (hardcoded per spec)
B, C, T, S = 16, 512, 2048, 128
NCORES = 8
BL = B // NCORES          # batches per core
W_LEN = 37
PAD = W_LEN // 2          # 18
EPS = 1e-9
SLOPE = 0.2
SQRT2 = 1.4142135623730951
G = C // 128              # 4 channel groups
NT = T // 512             # 4 time chunks
P0W = T + 2 * PAD         # 2084


def _build(nc, fc1b_nz, fc2b_nz, cb2_nz, alpha1, alpha2):
    """Build the per-core Tile program. Shapes are per-core (BL batches)."""
    xs_d = nc.dram_tensor("xs", [BL, G, 128, T], BF16, kind="ExternalInput").ap()
    s_d = nc.dram_tensor("s", [BL, 128, T], F32, kind="ExternalInput").ap()
    a_d = nc.dram_tensor("a", [BL, T], BF16, kind="ExternalInput").ap()
    fc1w_d = nc.dram_tensor("fc1w", [128, 2 * C], BF16, kind="ExternalInput").ap()
    fc2w_d = nc.dram_tensor("fc2w", [128, 2 * C], BF16, kind="ExternalInput").ap()
    c1w_d = nc.dram_tensor("c1w", [128, 3, G, C], BF16, kind="ExternalInput").ap()
    c2w_d = nc.dram_tensor("c2w", [128, 3, G, C], BF16, kind="ExternalInput").ap()
    ab2_d = nc.dram_tensor("ab2", [G, 128], F32, kind="ExternalInput").ap()   # alpha2*conv1_b tiled
    cb2_d = nc.dram_tensor("cb2", [G, 128], F32, kind="ExternalInput").ap()   # conv2_b/sqrt2 tiled
    # cnt*A rows for the (normally absent) fc-bias path
    ca_d = nc.dram_tensor("ca", [BL, T], BF16, kind="ExternalInput").ap()
    fb1_d = nc.dram_tensor("fb1", [1, 2 * C], BF16, kind="ExternalInput").ap()
    fb2_d = nc.dram_tensor("fb2", [1, 2 * C], BF16, kind="ExternalInput").ap()
    out_d = nc.dram_tensor("out", [BL, G, 128, T], F32, kind="ExternalOutput").ap()

    dma = nc.sync.dma_start

    with tile.TileContext(nc) as tc:
        with (
            tc.tile_pool(name="wpool", bufs=1) as wpool,
            tc.tile_pool(name="batch", bufs=2) as pb,
            tc.tile_pool(name="tree", bufs=2) as tr,
            tc.tile_pool(name="small", bufs=6) as sm,
            tc.tile_pool(name="pg", bufs=4, space="PSUM") as pg,
            tc.tile_pool(name="pc", bufs=4, space="PSUM") as pc,
        ):
            # ---- DMA priority order: batch-0 style inputs first (they gate
            # the windowed-sum tree, which gates everything), then fc1 weights,
            # then batch-0 x, then conv weights, then batch-1 inputs. ----
            abc0 = pb.tile([128, T], BF16, name="abc0", tag="abc")
            dma(out=abc0[:, :], in_=bass.AP(tensor=a_d.tensor, offset=a_d.offset,
                                            ap=[[0, 128], [1, T]]))
            p00 = tr.tile([128, P0W], F32, name="p00", tag="p0")
            nc.vector.memset(p00[:, 0:PAD], 0.0)
            nc.vector.memset(p00[:, T + PAD:P0W], 0.0)
            # split so the first tree chunk's slice lands first
            for (c0, c1) in ((0, 530), (530, 1042), (1042, 1554), (1554, 2048)):
                dma(out=p00[:, PAD + c0:PAD + c1], in_=s_d[0, :, c0:c1])

            # PE warm-up: dummy matmuls bridge the tree latency and flip the
            # HAM clock gate to 2.4 GHz before the real matmul stream starts.
            def warmup(tag, cnt, rhs=None, lhsT=None, uid=[0]):
                for _ in range(cnt):
                    uid[0] += 1
                    pwu = pc.tile([128, 512], F32, name=f"pwu{uid[0]}", tag=tag)
                    nc.tensor.matmul(pwu[:, :],
                                     lhsT if lhsT is not None else abc0[:, 0:128],
                                     rhs if rhs is not None else abc0[:, 0:512],
                                     start=True, stop=True)

            warmup("pc", 14)

            fc1w = wpool.tile([128, 2 * C], BF16, name="fc1w")
            dma(out=fc1w[:, :], in_=fc1w_d[:, :])
            xs0 = pb.tile([128, G, T], BF16, name="xs0", tag="xs")
            # chunks 0..1 of every channel group land first (they gate epi0/epi1)
            for g_ in range(G):
                dma(out=xs0[:, g_, 0:1024], in_=xs_d[0, g_, :, 0:1024])
            for g_ in range(G):
                dma(out=xs0[:, g_, 1024:T], in_=xs_d[0, g_, :, 1024:T])
            c1w = wpool.tile([128, 3, G, C], BF16, name="c1w")
            dma(out=c1w[:, :, :, :], in_=c1w_d[:, :, :, :])
            fc2w = wpool.tile([128, 2 * C], BF16, name="fc2w")
            dma(out=fc2w[:, :], in_=fc2w_d[:, :])
            c2w = wpool.tile([128, 3, G, C], BF16, name="c2w")
            dma(out=c2w[:, :, :, :], in_=c2w_d[:, :, :, :])
            ab2 = wpool.tile([128, G], F32, name="ab2")
            dma(out=ab2[:, :], in_=bass.AP(tensor=ab2_d.tensor, offset=ab2_d.offset,
                                           ap=[[1, 128], [128, G]]))
            cb2 = wpool.tile([128, G], F32, name="cb2")
            dma(out=cb2[:, :], in_=bass.AP(tensor=cb2_d.tensor, offset=cb2_d.offset,
                                           ap=[[1, 128], [128, G]]))
            if fc1b_nz or fc2b_nz:
                fb1 = wpool.tile([1, 2 * C], BF16, name="fb1")
                dma(out=fb1[:, :], in_=fb1_d[:, :])
                fb2 = wpool.tile([1, 2 * C], BF16, name="fb2")
                dma(out=fb2[:, :], in_=fb2_d[:, :])

            for b in range(BL):
                # ---- load batch inputs ----
                if b == 0:
                    abc, p0 = abc0, p00
                else:
                    abc = pb.tile([128, T], BF16, name=f"abc{b}", tag="abc")
                    dma(out=abc[:, :], in_=bass.AP(tensor=a_d.tensor,
                                                   offset=a_d.offset + b * T,
                                                   ap=[[0, 128], [1, T]]))
                    p0 = tr.tile([128, P0W], F32, name=f"p0{b}", tag="p0")
                    nc.vector.memset(p0[:, 0:PAD], 0.0)
                    nc.vector.memset(p0[:, T + PAD:P0W], 0.0)
                    dma(out=p0[:, PAD:T + PAD], in_=s_d[b, :, :])
                if b == 0:
                    xs = xs0
                else:
                    xs = pb.tile([128, G, T], BF16, name=f"xs{b}", tag="xs")
                    dma(out=xs[:, :, :], in_=xs_d[b, :, :, :].rearrange("g p t -> p g t"))
                if fc1b_nz or fc2b_nz:
                    ca = pb.tile([1, T], BF16, name=f"ca{b}", tag="ca")
                    dma(out=ca[:, :], in_=ca_d[b:b + 1, :])

                # ---- windowed-sum tree chunk: p0 cols [o, o+CW) -> wsa[:, o:o+512) ----
                wsa = pb.tile([128, T], BF16, name=f"wsa{b}", tag="wsa")
                CW = 512 + 36  # chunk input width in p0 coords

                def tree_chunk(n):
                    o = n * 512
                    pin = p0[:, o:o + CW]
                    t2 = tr.tile([128, CW - 1], BF16, name=f"t2_{b}_{n}", tag="t2")
                    nc.vector.tensor_add(t2[:, :], pin[:, 0:CW - 1], pin[:, 1:CW])
                    t4 = tr.tile([128, CW - 3], BF16, name=f"t4_{b}_{n}", tag="t4")
                    nc.vector.tensor_add(t4[:, :], t2[:, 0:CW - 3], t2[:, 2:CW - 1])
                    t8 = tr.tile([128, CW - 7], BF16, name=f"t8_{b}_{n}", tag="t8")
                    nc.vector.tensor_add(t8[:, :], t4[:, 0:CW - 7], t4[:, 4:CW - 3])
                    t16 = tr.tile([128, CW - 15], BF16, name=f"t16_{b}_{n}", tag="t16")
                    nc.vector.tensor_add(t16[:, :], t8[:, 0:CW - 15], t8[:, 8:CW - 7])
                    t32 = tr.tile([128, CW - 31], BF16, name=f"t32_{b}_{n}", tag="t32")
                    nc.vector.tensor_add(t32[:, :], t16[:, 0:CW - 31], t16[:, 16:CW - 15])
                    t36 = tr.tile([128, CW - 35], BF16, name=f"t36_{b}_{n}", tag="t36")
                    nc.vector.tensor_add(t36[:, :], t32[:, 0:CW - 35], t4[:, 32:CW - 3])
                    ws = tr.tile([128, 512], BF16, name=f"ws{b}_{n}", tag="ws")
                    nc.vector.tensor_add(ws[:, :], t36[:, 0:512], pin[:, 36:CW])
                    nc.vector.tensor_mul(wsa[:, o:o + 512], ws[:, :], abc[:, o:o + 512])

                if b > 0:
                    for _n in range(NT):
                        tree_chunk(_n)

                # ---- stage 1: g1 matmuls + epilogue -> h1 ----
                h1 = pb.tile([128, G, T + 4], BF16, name=f"h1_{b}", tag="h1")
                h2 = pb.tile([128, G, T + 4], BF16, name=f"h2_{b}", tag="h2", bufs=1)
                for m in range(G):
                    nc.vector.memset(h1[:, m, 0:2], 0.0)
                    nc.vector.memset(h1[:, m, T + 2:T + 4], 0.0)
                    nc.vector.memset(h2[:, m, 0:2], 0.0)
                    nc.vector.memset(h2[:, m, T + 2:T + 4], 0.0)

                def adawin_tile(m, n, fcw, fbt, fb_nz, xn_tile, h_out, uid,
                                fast_ramp=False):
                    """gamma/beta matmuls + (1+g)*xn + b epilogue + lrelu -> h_out chunk."""
                    ns = slice(n * 512, (n + 1) * 512)
                    pga = pg.tile([128, 512], F32, name=f"pga{uid}", tag="pg")
                    pgb = pg.tile([128, 512], F32, name=f"pgb{uid}", tag="pg")
                    if fb_nz:
                        nc.tensor.matmul(pga[:, :], fbt[:, m * 128:(m + 1) * 128],
                                         ca[:, ns], start=True, stop=False)
                        nc.tensor.matmul(pga[:, :], fcw[:, m * 128:(m + 1) * 128],
                                         wsa[:, ns], start=False, stop=True)
                        nc.tensor.matmul(pgb[:, :], fbt[:, C + m * 128:C + (m + 1) * 128],
                                         ca[:, ns], start=True, stop=False)
                        nc.tensor.matmul(pgb[:, :], fcw[:, C + m * 128:C + (m + 1) * 128],
                                         wsa[:, ns], start=False, stop=True)
                    else:
                        nc.tensor.matmul(pga[:, :], fcw[:, m * 128:(m + 1) * 128],
                                         wsa[:, ns], start=True, stop=True)
                        nc.tensor.matmul(pgb[:, :], fcw[:, C + m * 128:C + (m + 1) * 128],
                                         wsa[:, ns], start=True, stop=True)
                    if fast_ramp:
                        # ACT evacuates the gamma bank and folds the +1; keeps
                        # the ramp's serial DVE chain short (ACT is idle here).
                        cg = sm.tile([128, 512], BF16, name=f"cg{uid}", tag="u")
                        nc.scalar.activation(cg[:, :], pga[:, :], AF.Identity,
                                             bias=1.0)
                        u = sm.tile([128, 512], BF16, name=f"uf{uid}", tag="v")
                        nc.vector.tensor_mul(u[:, :], cg[:, :], xn_tile[:, :])
                        w = sm.tile([128, 512], BF16, name=f"w{uid}", tag="w")
                        nc.vector.tensor_add(w[:, :], u[:, :], pgb[:, :])
                    else:
                        u = sm.tile([128, 512], BF16, name=f"u{uid}", tag="u")
                        nc.vector.tensor_mul(u[:, :], pga[:, :], xn_tile[:, :])
                        v = sm.tile([128, 512], BF16, name=f"v{uid}", tag="v")
                        nc.vector.tensor_add(v[:, :], u[:, :], xn_tile[:, :])
                        w = sm.tile([128, 512], BF16, name=f"w{uid}", tag="w")
                        nc.vector.tensor_add(w[:, :], v[:, :], pgb[:, :])
                    nc.scalar.activation(h_out[:, m, 2 + n * 512:2 + (n + 1) * 512],
                                         w[:, :], AF.Prelu, alpha=SLOPE)

                def stage1_chunk(n):
                    for m in range(G):
                        xn = sm.tile([128, 512], BF16, name=f"xn1_{b}_{m}_{n}", tag="xn1")
                        nc.scalar.activation(xn, xs[:, m, n * 512:(n + 1) * 512],
                                             AF.Tanh, scale=alpha1 * SQRT2)
                        adawin_tile(m, n, fc1w, fb1 if fc1b_nz else None,
                                    fc1b_nz, xn, h1, f"1_{b}_{m}_{n}",
                                    fast_ramp=(n < 2))
                    if b == 0 and n < 2:
                        warmup("pc", 10, rhs=wsa[:, n * 512:(n + 1) * 512])

                if b == 0:
                    tree_chunk(0)
                    tree_chunk(1)
                    stage1_chunk(0)
                    stage1_chunk(1)
                    tree_chunk(2)
                    tree_chunk(3)
                    stage1_chunk(2)
                    stage1_chunk(3)
                else:
                    for n in range(NT):
                        stage1_chunk(n)

                # ---- conv1 -> tanh -> stage 2 -> h2 ----
                for n in range(NT):
                    for m in range(G):
                        pct = pc.tile([128, 512], F32, name=f"pc1_{b}_{m}_{n}", tag="pc")
                        for k in range(3):
                            for ki in range(G):
                                lhsT = c1w[:, k, ki, m * 128:(m + 1) * 128]
                                rhs = h1[:, ki, 1 + k + n * 512:1 + k + (n + 1) * 512]
                                nc.tensor.matmul(pct[:, :], lhsT, rhs,
                                                 start=(k == 0 and ki == 0),
                                                 stop=(k == 2 and ki == G - 1))
                        xn2 = sm.tile([128, 512], BF16, name=f"xn2_{b}_{m}_{n}", tag="xn2")
                        nc.scalar.activation(xn2, pct[:, :], AF.Tanh,
                                             bias=ab2[:, m:m + 1], scale=alpha2)
                        adawin_tile(m, n, fc2w, None if not fc2b_nz else fb2,
                                    fc2b_nz, xn2, h2, f"2_{b}_{m}_{n}")

                # ---- conv2 + residual -> out ----
                for n in range(NT):
                    for m in range(G):
                        pct = pc.tile([128, 512], F32, name=f"pc2_{b}_{m}_{n}", tag="pc")
                        for k in range(3):
                            for ki in range(G):
                                lhsT = c2w[:, k, ki, m * 128:(m + 1) * 128]
                                rhs = h2[:, ki, 1 + k + n * 512:1 + k + (n + 1) * 512]
                                nc.tensor.matmul(pct[:, :], lhsT, rhs,
                                                 start=(k == 0 and ki == 0),
                                                 stop=(k == 2 and ki == G - 1))
                        ot = sm.tile([128, 512], F32, name=f"ot{b}_{m}_{n}", tag="ot")
                        nc.vector.tensor_add(ot[:, :], pct[:, :],
                                             xs[:, m, n * 512:(n + 1) * 512])
                        if cb2_nz:
                            nc.scalar.activation(ot[:, :], ot[:, :], AF.Identity,
                                                 bias=cb2[:, m:m + 1])
                        dma(out=out_d[b, m, :, n * 512:(n + 1) * 512], in_=ot[:, :])
    return nc


_CACHE = {}


def _get_nc(fc1b_nz, fc2b_nz, cb2_nz, alpha1, alpha2):
    key = (fc1b_nz, fc2b_nz, cb2_nz, float(alpha1), float(alpha2))
    if key not in _CACHE:
        nc = bacc.Bacc("TRN2", target_bir_lowering=False, debug=False,
                       num_devices=NCORES)
        _build(nc, fc1b_nz, fc2b_nz, cb2_nz, alpha1, alpha2)
        nc.compile()
        _CACHE[key] = nc
    return _CACHE[key]


def _host_prep(x, s, lengths, fc1_w, fc1_b, alpha1, conv1_w, conv1_b,
               fc2_w, fc2_b, alpha2, conv2_w, conv2_b):
    """Host-side input preparation. Returns (in_maps, meta)."""
    x = np.asarray(x, np.float32)
    s = np.asarray(s, np.float32)
    lengths = np.asarray(lengths)
    a1 = float(np.asarray(alpha1).reshape(-1)[0])
    a2 = float(np.asarray(alpha2).reshape(-1)[0])

    # A = mask / (win_sum(mask) + eps), cnt = win_sum(ones)  -- all [B, T]
    t_idx = np.arange(T)
    mask = (t_idx[None, :] < lengths[:, None]).astype(np.float64)
    kern = np.ones(W_LEN)
    den = np.stack([np.convolve(mask[i], kern, mode="same") for i in range(B)]) + EPS
    A = (mask / den).astype(ml_dtypes.bfloat16)
    cnt = np.convolve(np.ones(T), kern, mode="same")
    cA = (A * cnt[None, :]).astype(ml_dtypes.bfloat16)

    bf = ml_dtypes.bfloat16
    xs = (x / SQRT2).reshape(B, G, 128, T).astype(bf)
    fc1wT = np.ascontiguousarray(fc1_w.T).astype(bf)             # [S, 2C]
    fc2wT = np.ascontiguousarray(fc2_w.T).astype(bf)
    # conv weights: [O, I, 3] -> [p=i%128, k, ki=i//128, o]
    c1wT = np.ascontiguousarray(
        conv1_w.astype(np.float32).transpose(1, 2, 0).reshape(G, 128, 3, C)
        .transpose(1, 2, 0, 3)).astype(bf)
    c2wT = np.ascontiguousarray(
        (conv2_w.astype(np.float32) / SQRT2).transpose(1, 2, 0).reshape(G, 128, 3, C)
        .transpose(1, 2, 0, 3)).astype(bf)
    ab2 = (a2 * conv1_b.astype(np.float32)).reshape(G, 128)
    cb2 = (conv2_b.astype(np.float32) / SQRT2).reshape(G, 128)
    fb1 = fc1_b.astype(bf).reshape(1, 2 * C)
    fb2 = fc2_b.astype(bf).reshape(1, 2 * C)
    fc1b_nz = bool(np.any(fc1_b))
    fc2b_nz = bool(np.any(fc2_b))
    cb2_nz = bool(np.any(conv2_b))

    in_maps = []
    for c in range(NCORES):
        bs = slice(c * BL, (c + 1) * BL)
        in_maps.append({
            "xs": xs[bs], "s": s[bs], "a": A[bs], "ca": cA[bs],
            "fc1w": fc1wT, "fc2w": fc2wT, "c1w": c1wT, "c2w": c2wT,
            "ab2": ab2, "cb2": cb2, "fb1": fb1, "fb2": fb2,
        })
    return in_maps, (fc1b_nz, fc2b_nz, cb2_nz, a1, a2)


def kernel(x, s, lengths, fc1_w, fc1_b, alpha1, conv1_w, conv1_b,
           fc2_w, fc2_b, alpha2, conv2_w, conv2_b, _trace=False):
    in_maps, (fc1b_nz, fc2b_nz, cb2_nz, a1, a2) = _host_prep(
        x, s, lengths, fc1_w, fc1_b, alpha1, conv1_w, conv1_b,
        fc2_w, fc2_b, alpha2, conv2_w, conv2_b)
    nc = _get_nc(fc1b_nz, fc2b_nz, cb2_nz, a1, a2)
    res = run_bass_kernel_spmd(nc, in_maps, core_ids=list(range(NCORES)),
                               trace=_trace)
    out = np.concatenate([res.results[i]["out"] for i in range(NCORES)], axis=0)
    out = out.reshape(B, C, T).astype(np.float32)
    kernel.last_exec_time_ns = res.exec_time_ns
    return out


# revision 39
# speedup vs baseline: 1.0142x; 1.0062x over previous
"""AdaWinBlock1d Trainium2 kernel — 8 NeuronCores, data-parallel over batch.

Reference computation (per batch, C=512, T=2048, S=128, W=37):
    mask = t < length                       [T]
    A    = mask / (win_sum(mask) + eps)     [T]       (host, from lengths only)
    ws   = win_sum(s)                       [S, T]    (device, shift-add tree)
    g    = fc_w @ (ws * A)                  [2C, T]   (win_sum linearity + column scaling)
    xn   = tanh(alpha * x)
    y    = (1 + g_gamma) * xn + g_beta
    h    = leaky_relu(y, 0.2)
    c    = conv1d(h, w, b)   (kernel 3, pad 1; 3 shifted matmuls in PSUM)
    ... second adawin + conv ...
    out  = (c2 + x) / sqrt(2)               (1/sqrt2 folded into conv2 weights & pre-scaled x)
"""

import sys, types, os

sys.path.insert(0, '/opt/trn_rl_repo')

# ---------------------------------------------------------------------------
# Shim antenv.axon_hooks (missing in this image) so trace=True works.
# ---------------------------------------------------------------------------
if "antenv.axon_hooks" not in sys.modules:
    _m = types.ModuleType("antenv.axon_hooks")
    _m._hook = None
    def _set_hook(h):
        _m._hook = h
    def _get_hook():
        return _m._hook
    _m.set_axon_ntff_profile_hook = _set_hook
    _m.get_axon_ntff_profile_hook = _get_hook
    sys.modules["antenv.axon_hooks"] = _m
    try:
        import antenv
        antenv.axon_hooks = _m
        from trn_agent_boot.trn_boot import _ntff_profile_via_ctypes
        hook = _ntff_profile_via_ctypes('/opt/axon/libaxon_pjrt.so')
        if hook is not None:
            _set_hook(hook)
    except Exception:
        pass

import numpy as np
import ml_dtypes

import concourse.bass_utils as bass_utils
bass_utils.upload_artifacts = lambda tmpdir: tmpdir  # no cloud store here

import concourse.bass as bass
import concourse.tile as tile
from concourse import mybir, bacc
from concourse.bass_utils import run_bass_kernel_spmd

F32 = mybir.dt.float32
BF16 = mybir.dt.bfloat16
AF = mybir.ActivationFunctionType

# Problem constants (hardcoded per spec)
B, C, T, S = 16, 512, 2048, 128
NCORES = 8
BL = B // NCORES          # batches per core
W_LEN = 37
PAD = W_LEN // 2          # 18
EPS = 1e-9
SLOPE = 0.2
SQRT2 = 1.4142135623730951
G = C // 128              # 4 channel groups
NT = T // 512             # 4 time chunks
P0W = T + 2 * PAD         # 2084


def _build(nc, fc1b_nz, fc2b_nz, cb2_nz, alpha1, alpha2):
    """Build the per-core Tile program. Shapes are per-core (BL batches)."""
    xs_d = nc.dram_tensor("xs", [BL, G, 128, T], BF16, kind="ExternalInput").ap()
    s_d = nc.dram_tensor("s", [BL, 128, T], BF16, kind="ExternalInput").ap()
    a_d = nc.dram_tensor("a", [BL, T], BF16, kind="ExternalInput").ap()
    fc1w_d = nc.dram_tensor("fc1w", [128, 2 * C], BF16, kind="ExternalInput").ap()
    fc2w_d = nc.dram_tensor("fc2w", [128, 2 * C], BF16, kind="ExternalInput").ap()
    c1w_d = nc.dram_tensor("c1w", [128, 3, G, C], BF16, kind="ExternalInput").ap()
    c2w_d = nc.dram_tensor("c2w", [128, 3, G, C], BF16, kind="ExternalInput").ap()
    ab2_d = nc.dram_tensor("ab2", [G, 128], F32, kind="ExternalInput").ap()   # alpha2*conv1_b tiled
    cb2_d = nc.dram_tensor("cb2", [G, 128], F32, kind="ExternalInput").ap()   # conv2_b/sqrt2 tiled
    # cnt*A rows for the (normally absent) fc-bias path
    ca_d = nc.dram_tensor("ca", [BL, T], BF16, kind="ExternalInput").ap()
    fb1_d = nc.dram_tensor("fb1", [1, 2 * C], BF16, kind="ExternalInput").ap()
    fb2_d = nc.dram_tensor("fb2", [1, 2 * C], BF16, kind="ExternalInput").ap()
    out_d = nc.dram_tensor("out", [BL, G, 128, T], F32, kind="ExternalOutput").ap()

    dma = nc.sync.dma_start

    with tile.TileContext(nc) as tc:
        with (
            tc.tile_pool(name="wpool", bufs=1) as wpool,
            tc.tile_pool(name="batch", bufs=2) as pb,
            tc.tile_pool(name="tree", bufs=2) as tr,
            tc.tile_pool(name="small", bufs=6) as sm,
            tc.tile_pool(name="pg", bufs=4, space="PSUM") as pg,
            tc.tile_pool(name="pc", bufs=4, space="PSUM") as pc,
        ):
            # ---- DMA priority order: batch-0 style inputs first (they gate
            # the windowed-sum tree, which gates everything), then fc1 weights,
            # then batch-0 x, then conv weights, then batch-1 inputs. ----
            abc0 = pb.tile([128, T], BF16, name="abc0", tag="abc")
            dma(out=abc0[:, :], in_=bass.AP(tensor=a_d.tensor, offset=a_d.offset,
                                            ap=[[0, 128], [1, T]]))
            p00 = tr.tile([128, P0W], BF16, name="p00", tag="p0")
            nc.vector.memset(p00[:, 0:PAD], 0.0)
            nc.vector.memset(p00[:, T + PAD:P0W], 0.0)
            # split so the first tree chunk's slice lands first
            for (c0, c1) in ((0, 530), (530, 1042), (1042, 1554), (1554, 2048)):
                dma(out=p00[:, PAD + c0:PAD + c1], in_=s_d[0, :, c0:c1])

            # PE warm-up: dummy matmuls bridge the tree latency and flip the
            # HAM clock gate to 2.4 GHz before the real matmul stream starts.
            def warmup(tag, cnt, rhs=None, lhsT=None, uid=[0]):
                for _ in range(cnt):
                    uid[0] += 1
                    pwu = pc.tile([128, 512], F32, name=f"pwu{uid[0]}", tag=tag)
                    nc.tensor.matmul(pwu[:, :],
                                     lhsT if lhsT is not None else abc0[:, 0:128],
                                     rhs if rhs is not None else abc0[:, 0:512],
                                     start=True, stop=True)

            warmup("pc", 14)

            fc1w = wpool.tile([128, 2 * C], BF16, name="fc1w")
            dma(out=fc1w[:, :], in_=fc1w_d[:, :])
            xs0 = pb.tile([128, G, T], BF16, name="xs0", tag="xs")
            # chunks 0..1 of every channel group land first (they gate epi0/epi1)
            for g_ in range(G):
                dma(out=xs0[:, g_, 0:1024], in_=xs_d[0, g_, :, 0:1024])
            for g_ in range(G):
                dma(out=xs0[:, g_, 1024:T], in_=xs_d[0, g_, :, 1024:T])
            c1w = wpool.tile([128, 3, G, C], BF16, name="c1w")
            dma(out=c1w[:, :, :, :], in_=c1w_d[:, :, :, :])
            fc2w = wpool.tile([128, 2 * C], BF16, name="fc2w")
            dma(out=fc2w[:, :], in_=fc2w_d[:, :])
            c2w = wpool.tile([128, 3, G, C], BF16, name="c2w")
            dma(out=c2w[:, :, :, :], in_=c2w_d[:, :, :, :])
            ab2 = wpool.tile([128, G], F32, name="ab2")
            dma(out=ab2[:, :], in_=bass.AP(tensor=ab2_d.tensor, offset=ab2_d.offset,
                                           ap=[[1, 128], [128, G]]))
            cb2 = wpool.tile([128, G], F32, name="cb2")
            dma(out=cb2[:, :], in_=bass.AP(tensor=cb2_d.tensor, offset=cb2_d.offset,
                                           ap=[[1, 128], [128, G]]))
            if fc1b_nz or fc2b_nz:
                fb1 = wpool.tile([1, 2 * C], BF16, name="fb1")
                dma(out=fb1[:, :], in_=fb1_d[:, :])
                fb2 = wpool.tile([1, 2 * C], BF16, name="fb2")
                dma(out=fb2[:, :], in_=fb2_d[:, :])

            for b in range(BL):
                # ---- load batch inputs ----
                if b == 0:
                    abc, p0 = abc0, p00
                else:
                    abc = pb.tile([128, T], BF16, name=f"abc{b}", tag="abc")
                    dma(out=abc[:, :], in_=bass.AP(tensor=a_d.tensor,
                                                   offset=a_d.offset + b * T,
                                                   ap=[[0, 128], [1, T]]))
                    p0 = tr.tile([128, P0W], BF16, name=f"p0{b}", tag="p0")
                    nc.vector.memset(p0[:, 0:PAD], 0.0)
                    nc.vector.memset(p0[:, T + PAD:P0W], 0.0)
                    dma(out=p0[:, PAD:T + PAD], in_=s_d[b, :, :])
                if b == 0:
                    xs = xs0
                else:
                    xs = pb.tile([128, G, T], BF16, name=f"xs{b}", tag="xs")
                    dma(out=xs[:, :, :], in_=xs_d[b, :, :, :].rearrange("g p t -> p g t"))
                if fc1b_nz or fc2b_nz:
                    ca = pb.tile([1, T], BF16, name=f"ca{b}", tag="ca")
                    dma(out=ca[:, :], in_=ca_d[b:b + 1, :])

                # ---- windowed-sum tree chunk: p0 cols [o, o+CW) -> wsa[:, o:o+512) ----
                wsa = pb.tile([128, T], BF16, name=f"wsa{b}", tag="wsa")
                CW = 512 + 36  # chunk input width in p0 coords

                def tree_chunk(n):
                    o = n * 512
                    pin = p0[:, o:o + CW]
                    t2 = tr.tile([128, CW - 1], BF16, name=f"t2_{b}_{n}", tag="t2")
                    nc.vector.tensor_add(t2[:, :], pin[:, 0:CW - 1], pin[:, 1:CW])
                    t4 = tr.tile([128, CW - 3], BF16, name=f"t4_{b}_{n}", tag="t4")
                    nc.vector.tensor_add(t4[:, :], t2[:, 0:CW - 3], t2[:, 2:CW - 1])
                    t8 = tr.tile([128, CW - 7], BF16, name=f"t8_{b}_{n}", tag="t8")
                    nc.vector.tensor_add(t8[:, :], t4[:, 0:CW - 7], t4[:, 4:CW - 3])
                    t16 = tr.tile([128, CW - 15], BF16, name=f"t16_{b}_{n}", tag="t16")
                    nc.vector.tensor_add(t16[:, :], t8[:, 0:CW - 15], t8[:, 8:CW - 7])
                    t32 = tr.tile([128, CW - 31], BF16, name=f"t32_{b}_{n}", tag="t32")
                    nc.vector.tensor_add(t32[:, :], t16[:, 0:CW - 31], t16[:, 16:CW - 15])
                    t36 = tr.tile([128, CW - 35], BF16, name=f"t36_{b}_{n}", tag="t36")
                    nc.vector.tensor_add(t36[:, :], t32[:, 0:CW - 35], t4[:, 32:CW - 3])
                    ws = tr.tile([128, 512], BF16, name=f"ws{b}_{n}", tag="ws")
                    nc.vector.tensor_add(ws[:, :], t36[:, 0:512], pin[:, 36:CW])
                    nc.vector.tensor_mul(wsa[:, o:o + 512], ws[:, :], abc[:, o:o + 512])

                if b > 0:
                    for _n in range(NT):
                        tree_chunk(_n)

                # ---- stage 1: g1 matmuls + epilogue -> h1 ----
                h1 = pb.tile([128, G, T + 4], BF16, name=f"h1_{b}", tag="h1")
                h2 = pb.tile([128, G, T + 4], BF16, name=f"h2_{b}", tag="h2", bufs=1)
                for m in range(G):
                    nc.vector.memset(h1[:, m, 0:2], 0.0)
                    nc.vector.memset(h1[:, m, T + 2:T + 4], 0.0)
                    nc.vector.memset(h2[:, m, 0:2], 0.0)
                    nc.vector.memset(h2[:, m, T + 2:T + 4], 0.0)

                def adawin_tile(m, n, fcw, fbt, fb_nz, xn_tile, h_out, uid,
                                fast_ramp=False):
                    """gamma/beta matmuls + (1+g)*xn + b epilogue + lrelu -> h_out chunk."""
                    ns = slice(n * 512, (n + 1) * 512)
                    pga = pg.tile([128, 512], F32, name=f"pga{uid}", tag="pg")
                    pgb = pg.tile([128, 512], F32, name=f"pgb{uid}", tag="pg")
                    if fb_nz:
                        nc.tensor.matmul(pga[:, :], fbt[:, m * 128:(m + 1) * 128],
                                         ca[:, ns], start=True, stop=False)
                        nc.tensor.matmul(pga[:, :], fcw[:, m * 128:(m + 1) * 128],
                                         wsa[:, ns], start=False, stop=True)
                        nc.tensor.matmul(pgb[:, :], fbt[:, C + m * 128:C + (m + 1) * 128],
                                         ca[:, ns], start=True, stop=False)
                        nc.tensor.matmul(pgb[:, :], fcw[:, C + m * 128:C + (m + 1) * 128],
                                         wsa[:, ns], start=False, stop=True)
                    else:
                        nc.tensor.matmul(pga[:, :], fcw[:, m * 128:(m + 1) * 128],
                                         wsa[:, ns], start=True, stop=True)
                        nc.tensor.matmul(pgb[:, :], fcw[:, C + m * 128:C + (m + 1) * 128],
                                         wsa[:, ns], start=True, stop=True)
                    if fast_ramp:
                        # ACT evacuates the gamma bank and folds the +1; keeps
                        # the ramp's serial DVE chain short (ACT is idle here).
                        cg = sm.tile([128, 512], BF16, name=f"cg{uid}", tag="u")
                        nc.scalar.activation(cg[:, :], pga[:, :], AF.Identity,
                                             bias=1.0)
                        u = sm.tile([128, 512], BF16, name=f"uf{uid}", tag="v")
                        nc.vector.tensor_mul(u[:, :], cg[:, :], xn_tile[:, :])
                        w = sm.tile([128, 512], BF16, name=f"w{uid}", tag="w")
                        nc.vector.tensor_add(w[:, :], u[:, :], pgb[:, :])
                    else:
                        u = sm.tile([128, 512], BF16, name=f"u{uid}", tag="u")
                        nc.vector.tensor_mul(u[:, :], pga[:, :], xn_tile[:, :])
                        v = sm.tile([128, 512], BF16, name=f"v{uid}", tag="v")
                        nc.vector.tensor_add(v[:, :], u[:, :], xn_tile[:, :])
                        w = sm.tile([128, 512], BF16, name=f"w{uid}", tag="w")
                        nc.vector.tensor_add(w[:, :], v[:, :], pgb[:, :])
                    nc.scalar.activation(h_out[:, m, 2 + n * 512:2 + (n + 1) * 512],
                                         w[:, :], AF.Prelu, alpha=SLOPE)

                def stage1_chunk(n):
                    for m in range(G):
                        xn = sm.tile([128, 512], BF16, name=f"xn1_{b}_{m}_{n}", tag="xn1")
                        nc.scalar.activation(xn, xs[:, m, n * 512:(n + 1) * 512],
                                             AF.Tanh, scale=alpha1 * SQRT2)
                        adawin_tile(m, n, fc1w, fb1 if fc1b_nz else None,
                                    fc1b_nz, xn, h1, f"1_{b}_{m}_{n}",
                                    fast_ramp=(n < 2))
                    if b == 0 and n < 2:
                        warmup("pc", 10, rhs=wsa[:, n * 512:(n + 1) * 512])

                if b == 0:
                    tree_chunk(0)
                    tree_chunk(1)
                    stage1_chunk(0)
                    stage1_chunk(1)
                    tree_chunk(2)
                    tree_chunk(3)
                    stage1_chunk(2)
                    stage1_chunk(3)
                else:
                    for n in range(NT):
                        stage1_chunk(n)

                # ---- conv1 -> tanh -> stage 2 -> h2 ----
                for n in range(NT):
                    for m in range(G):
                        pct = pc.tile([128, 512], F32, name=f"pc1_{b}_{m}_{n}", tag="pc")
                        for k in range(3):
                            for ki in range(G):
                                lhsT = c1w[:, k, ki, m * 128:(m + 1) * 128]
                                rhs = h1[:, ki, 1 + k + n * 512:1 + k + (n + 1) * 512]
                                nc.tensor.matmul(pct[:, :], lhsT, rhs,
                                                 start=(k == 0 and ki == 0),
                                                 stop=(k == 2 and ki == G - 1))
                        xn2 = sm.tile([128, 512], BF16, name=f"xn2_{b}_{m}_{n}", tag="xn2")
                        nc.scalar.activation(xn2, pct[:, :], AF.Tanh,
                                             bias=ab2[:, m:m + 1], scale=alpha2)
                        adawin_tile(m, n, fc2w, None if not fc2b_nz else fb2,
                                    fc2b_nz, xn2, h2, f"2_{b}_{m}_{n}")

                # ---- conv2 + residual -> out ----
                for n in range(NT):
                    for m in range(G):
                        pct = pc.tile([128, 512], F32, name=f"pc2_{b}_{m}_{n}", tag="pc")
                        for k in range(3):
                            for ki in range(G):
                                lhsT = c2w[:, k, ki, m * 128:(m + 1) * 128]
                                rhs = h2[:, ki, 1 + k + n * 512:1 + k + (n + 1) * 512]
                                nc.tensor.matmul(pct[:, :], lhsT, rhs,
                                                 start=(k == 0 and ki == 0),
                                                 stop=(k == 2 and ki == G - 1))
                        ot = sm.tile([128, 512], F32, name=f"ot{b}_{m}_{n}", tag="ot")
                        nc.vector.tensor_add(ot[:, :], pct[:, :],
                                             xs[:, m, n * 512:(n + 1) * 512])
                        if cb2_nz:
                            nc.scalar.activation(ot[:, :], ot[:, :], AF.Identity,
                                                 bias=cb2[:, m:m + 1])
                        dma(out=out_d[b, m, :, n * 512:(n + 1) * 512], in_=ot[:, :])
    return nc


_CACHE = {}


def _get_nc(fc1b_nz, fc2b_nz, cb2_nz, alpha1, alpha2):
    key = (fc1b_nz, fc2b_nz, cb2_nz, float(alpha1), float(alpha2))
    if key not in _CACHE:
        nc = bacc.Bacc("TRN2", target_bir_lowering=False, debug=False,
                       num_devices=NCORES)
        _build(nc, fc1b_nz, fc2b_nz, cb2_nz, alpha1, alpha2)
        nc.compile()
        _CACHE[key] = nc
    return _CACHE[key]


def _host_prep(x, s, lengths, fc1_w, fc1_b, alpha1, conv1_w, conv1_b,
               fc2_w, fc2_b, alpha2, conv2_w, conv2_b):
    """Host-side input preparation. Returns (in_maps, meta)."""
    x = np.asarray(x, np.float32)
    s = np.asarray(s).astype(ml_dtypes.bfloat16)
    lengths = np.asarray(lengths)
    a1 = float(np.asarray(alpha1).reshape(-1)[0])
    a2 = float(np.asarray(alpha2).reshape(-1)[0])

    # A = mask / (win_sum(mask) + eps), cnt = win_sum(ones)  -- all [B, T]
    t_idx = np.arange(T)
    mask = (t_idx[None, :] < lengths[:, None]).astype(np.float64)
    kern = np.ones(W_LEN)
    den = np.stack([np.convolve(mask[i], kern, mode="same") for i in range(B)]) + EPS
    A = (mask / den).astype(ml_dtypes.bfloat16)
    cnt = np.convolve(np.ones(T), kern, mode="same")
    cA = (A * cnt[None, :]).astype(ml_dtypes.bfloat16)

    bf = ml_dtypes.bfloat16
    xs = (x / SQRT2).reshape(B, G, 128, T).astype(bf)
    fc1wT = np.ascontiguousarray(fc1_w.T).astype(bf)             # [S, 2C]
    fc2wT = np.ascontiguousarray(fc2_w.T).astype(bf)
    # conv weights: [O, I, 3] -> [p=i%128, k, ki=i//128, o]
    c1wT = np.ascontiguousarray(
        conv1_w.astype(np.float32).transpose(1, 2, 0).reshape(G, 128, 3, C)
        .transpose(1, 2, 0, 3)).astype(bf)
    c2wT = np.ascontiguousarray(
        (conv2_w.astype(np.float32) / SQRT2).transpose(1, 2, 0).reshape(G, 128, 3, C)
        .transpose(1, 2, 0, 3)).astype(bf)
    ab2 = (a2 * conv1_b.astype(np.float32)).reshape(G, 128)
    cb2 = (conv2_b.astype(np.float32) / SQRT2).reshape(G, 128)
    fb1 = fc1_b.astype(bf).reshape(1, 2 * C)
    fb2 = fc2_b.astype(bf).reshape(1, 2 * C)
    fc1b_nz = bool(np.any(fc1_b))
    fc2b_nz = bool(np.any(fc2_b))
    cb2_nz = bool(np.any(conv2_b))

    in_maps = []
    for c in range(NCORES):
        bs = slice(c * BL, (c + 1) * BL)
        in_maps.append({
            "xs": xs[bs], "s": s[bs], "a": A[bs], "ca": cA[bs],
            "fc1w": fc1wT, "fc2w": fc2wT, "c1w": c1wT, "c2w": c2wT,
            "ab2": ab2, "cb2": cb2, "fb1": fb1, "fb2": fb2,
        })
    return in_maps, (fc1b_nz, fc2b_nz, cb2_nz, a1, a2)


def kernel(x, s, lengths, fc1_w, fc1_b, alpha1, conv1_w, conv1_b,
           fc2_w, fc2_b, alpha2, conv2_w, conv2_b, _trace=False):
    in_maps, (fc1b_nz, fc2b_nz, cb2_nz, a1, a2) = _host_prep(
        x, s, lengths, fc1_w, fc1_b, alpha1, conv1_w, conv1_b,
        fc2_w, fc2_b, alpha2, conv2_w, conv2_b)
    nc = _get_nc(fc1b_nz, fc2b_nz, cb2_nz, a1, a2)
    res = run_bass_kernel_spmd(nc, in_maps, core_ids=list(range(NCORES)),
                               trace=_trace)
    out = np.concatenate([res.results[i]["out"] for i in range(NCORES)], axis=0)
    out = out.reshape(B, C, T).astype(np.float32)
    kernel.last_exec_time_ns = res.exec_time_ns
    return out


# revision 40
# speedup vs baseline: 1.0195x; 1.0052x over previous
"""AdaWinBlock1d Trainium2 kernel — 8 NeuronCores, data-parallel over batch.

Reference computation (per batch, C=512, T=2048, S=128, W=37):
    mask = t < length                       [T]
    A    = mask / (win_sum(mask) + eps)     [T]       (host, from lengths only)
    ws   = win_sum(s)                       [S, T]    (device, shift-add tree)
    g    = fc_w @ (ws * A)                  [2C, T]   (win_sum linearity + column scaling)
    xn   = tanh(alpha * x)
    y    = (1 + g_gamma) * xn + g_beta
    h    = leaky_relu(y, 0.2)
    c    = conv1d(h, w, b)   (kernel 3, pad 1; 3 shifted matmuls in PSUM)
    ... second adawin + conv ...
    out  = (c2 + x) / sqrt(2)               (1/sqrt2 folded into conv2 weights & pre-scaled x)
"""

import sys, types, os

sys.path.insert(0, '/opt/trn_rl_repo')

# ---------------------------------------------------------------------------
# Shim antenv.axon_hooks (missing in this image) so trace=True works.
# ---------------------------------------------------------------------------
if "antenv.axon_hooks" not in sys.modules:
    _m = types.ModuleType("antenv.axon_hooks")
    _m._hook = None
    def _set_hook(h):
        _m._hook = h
    def _get_hook():
        return _m._hook
    _m.set_axon_ntff_profile_hook = _set_hook
    _m.get_axon_ntff_profile_hook = _get_hook
    sys.modules["antenv.axon_hooks"] = _m
    try:
        import antenv
        antenv.axon_hooks = _m
        from trn_agent_boot.trn_boot import _ntff_profile_via_ctypes
        hook = _ntff_profile_via_ctypes('/opt/axon/libaxon_pjrt.so')
        if hook is not None:
            _set_hook(hook)
    except Exception:
        pass

import numpy as np
import ml_dtypes

import concourse.bass_utils as bass_utils
bass_utils.upload_artifacts = lambda tmpdir: tmpdir  # no cloud store here

import concourse.bass as bass
import concourse.tile as tile
from concourse import mybir, bacc
from concourse.bass_utils import run_bass_kernel_spmd

F32 = mybir.dt.float32
BF16 = mybir.dt.bfloat16
AF = mybir.ActivationFunctionType

# Problem constants (hardcoded per spec)
B, C, T, S = 16, 512, 2048, 128
NCORES = 8
BL = B // NCORES          # batches per core
W_LEN = 37
PAD = W_LEN // 2          # 18
EPS = 1e-9
SLOPE = 0.2
SQRT2 = 1.4142135623730951
G = C // 128              # 4 channel groups
NT = T // 512             # 4 time chunks
P0W = T + 2 * PAD         # 2084


def _build(nc, fc1b_nz, fc2b_nz, cb2_nz, alpha1, alpha2):
    """Build the per-core Tile program. Shapes are per-core (BL batches)."""
    xs_d = nc.dram_tensor("xs", [BL, G, 128, T], BF16, kind="ExternalInput").ap()
    s_d = nc.dram_tensor("s", [BL, 128, T], BF16, kind="ExternalInput").ap()
    a_d = nc.dram_tensor("a", [BL, T], BF16, kind="ExternalInput").ap()
    fc1w_d = nc.dram_tensor("fc1w", [128, 2 * C], BF16, kind="ExternalInput").ap()
    fc2w_d = nc.dram_tensor("fc2w", [128, 2 * C], BF16, kind="ExternalInput").ap()
    c1w_d = nc.dram_tensor("c1w", [128, 3, G, C], BF16, kind="ExternalInput").ap()
    c2w_d = nc.dram_tensor("c2w", [128, 3, G, C], BF16, kind="ExternalInput").ap()
    ab2_d = nc.dram_tensor("ab2", [G, 128], F32, kind="ExternalInput").ap()   # alpha2*conv1_b tiled
    cb2_d = nc.dram_tensor("cb2", [G, 128], F32, kind="ExternalInput").ap()   # conv2_b/sqrt2 tiled
    # cnt*A rows for the (normally absent) fc-bias path
    ca_d = nc.dram_tensor("ca", [BL, T], BF16, kind="ExternalInput").ap()
    fb1_d = nc.dram_tensor("fb1", [1, 2 * C], BF16, kind="ExternalInput").ap()
    fb2_d = nc.dram_tensor("fb2", [1, 2 * C], BF16, kind="ExternalInput").ap()
    out_d = nc.dram_tensor("out", [BL, G, 128, T], F32, kind="ExternalOutput").ap()

    dma = nc.sync.dma_start

    with tile.TileContext(nc) as tc:
        with (
            tc.tile_pool(name="wpool", bufs=1) as wpool,
            tc.tile_pool(name="batch", bufs=2) as pb,
            tc.tile_pool(name="tree", bufs=2) as tr,
            tc.tile_pool(name="small", bufs=6) as sm,
            tc.tile_pool(name="pg", bufs=4, space="PSUM") as pg,
            tc.tile_pool(name="pc", bufs=4, space="PSUM") as pc,
        ):
            # ---- DMA priority order: batch-0 style inputs first (they gate
            # the windowed-sum tree, which gates everything), then fc1 weights,
            # then batch-0 x, then conv weights, then batch-1 inputs. ----
            abc0 = pb.tile([128, T], BF16, name="abc0", tag="abc")
            dma(out=abc0[:, :], in_=bass.AP(tensor=a_d.tensor, offset=a_d.offset,
                                            ap=[[0, 128], [1, T]]))
            p00 = tr.tile([128, P0W], BF16, name="p00", tag="p0")
            nc.vector.memset(p00[:, 0:PAD], 0.0)
            nc.vector.memset(p00[:, T + PAD:P0W], 0.0)
            # split so the first tree chunk's slice lands first
            for (c0, c1) in ((0, 530), (530, 1042), (1042, 1554), (1554, 2048)):
                dma(out=p00[:, PAD + c0:PAD + c1], in_=s_d[0, :, c0:c1])

            # PE warm-up: dummy matmuls bridge the tree latency and flip the
            # HAM clock gate to 2.4 GHz before the real matmul stream starts.
            def warmup(tag, cnt, rhs=None, lhsT=None, uid=[0]):
                for _ in range(cnt):
                    uid[0] += 1
                    pwu = pc.tile([128, 512], F32, name=f"pwu{uid[0]}", tag=tag)
                    nc.tensor.matmul(pwu[:, :],
                                     lhsT if lhsT is not None else abc0[:, 0:128],
                                     rhs if rhs is not None else abc0[:, 0:512],
                                     start=True, stop=True)

            warmup("pc", 14)

            fc1w = wpool.tile([128, 2 * C], BF16, name="fc1w")
            dma(out=fc1w[:, :], in_=fc1w_d[:, :])
            xs0 = pb.tile([128, G, T], BF16, name="xs0", tag="xs")
            # chunks 0..1 of every channel group land first (they gate epi0/epi1)
            for g_ in range(G):
                dma(out=xs0[:, g_, 0:1024], in_=xs_d[0, g_, :, 0:1024])
            for g_ in range(G):
                dma(out=xs0[:, g_, 1024:T], in_=xs_d[0, g_, :, 1024:T])
            c1w = wpool.tile([128, 3, G, C], BF16, name="c1w")
            dma(out=c1w[:, :, :, :], in_=c1w_d[:, :, :, :])
            fc2w = wpool.tile([128, 2 * C], BF16, name="fc2w")
            dma(out=fc2w[:, :], in_=fc2w_d[:, :])
            c2w = wpool.tile([128, 3, G, C], BF16, name="c2w")
            dma(out=c2w[:, :, :, :], in_=c2w_d[:, :, :, :])
            ab2 = wpool.tile([128, G], F32, name="ab2")
            dma(out=ab2[:, :], in_=bass.AP(tensor=ab2_d.tensor, offset=ab2_d.offset,
                                           ap=[[1, 128], [128, G]]))
            cb2 = wpool.tile([128, G], F32, name="cb2")
            dma(out=cb2[:, :], in_=bass.AP(tensor=cb2_d.tensor, offset=cb2_d.offset,
                                           ap=[[1, 128], [128, G]]))
            if fc1b_nz or fc2b_nz:
                fb1 = wpool.tile([1, 2 * C], BF16, name="fb1")
                dma(out=fb1[:, :], in_=fb1_d[:, :])
                fb2 = wpool.tile([1, 2 * C], BF16, name="fb2")
                dma(out=fb2[:, :], in_=fb2_d[:, :])

            for b in range(BL):
                # ---- load batch inputs ----
                if b == 0:
                    abc, p0 = abc0, p00
                else:
                    abc = pb.tile([128, T], BF16, name=f"abc{b}", tag="abc")
                    dma(out=abc[:, :], in_=bass.AP(tensor=a_d.tensor,
                                                   offset=a_d.offset + b * T,
                                                   ap=[[0, 128], [1, T]]))
                    p0 = tr.tile([128, P0W], BF16, name=f"p0{b}", tag="p0")
                    nc.vector.memset(p0[:, 0:PAD], 0.0)
                    nc.vector.memset(p0[:, T + PAD:P0W], 0.0)
                    dma(out=p0[:, PAD:T + PAD], in_=s_d[b, :, :])
                if b == 0:
                    xs = xs0
                else:
                    xs = pb.tile([128, G, T], BF16, name=f"xs{b}", tag="xs")
                    dma(out=xs[:, :, :], in_=xs_d[b, :, :, :].rearrange("g p t -> p g t"))
                if fc1b_nz or fc2b_nz:
                    ca = pb.tile([1, T], BF16, name=f"ca{b}", tag="ca")
                    dma(out=ca[:, :], in_=ca_d[b:b + 1, :])

                # ---- windowed-sum tree chunk: p0 cols [o, o+CW) -> wsa[:, o:o+512) ----
                wsa = pb.tile([128, T], BF16, name=f"wsa{b}", tag="wsa")
                CW = 512 + 36  # chunk input width in p0 coords

                def tree_chunk(n):
                    o = n * 512
                    pin = p0[:, o:o + CW]
                    t2 = tr.tile([128, CW - 1], BF16, name=f"t2_{b}_{n}", tag="t2")
                    nc.vector.tensor_add(t2[:, :], pin[:, 0:CW - 1], pin[:, 1:CW])
                    t4 = tr.tile([128, CW - 3], BF16, name=f"t4_{b}_{n}", tag="t4")
                    nc.vector.tensor_add(t4[:, :], t2[:, 0:CW - 3], t2[:, 2:CW - 1])
                    t8 = tr.tile([128, CW - 7], BF16, name=f"t8_{b}_{n}", tag="t8")
                    nc.vector.tensor_add(t8[:, :], t4[:, 0:CW - 7], t4[:, 4:CW - 3])
                    t16 = tr.tile([128, CW - 15], BF16, name=f"t16_{b}_{n}", tag="t16")
                    nc.vector.tensor_add(t16[:, :], t8[:, 0:CW - 15], t8[:, 8:CW - 7])
                    t32 = tr.tile([128, CW - 31], BF16, name=f"t32_{b}_{n}", tag="t32")
                    nc.vector.tensor_add(t32[:, :], t16[:, 0:CW - 31], t16[:, 16:CW - 15])
                    t36 = tr.tile([128, CW - 35], BF16, name=f"t36_{b}_{n}", tag="t36")
                    nc.vector.tensor_add(t36[:, :], t32[:, 0:CW - 35], t4[:, 32:CW - 3])
                    ws = tr.tile([128, 512], BF16, name=f"ws{b}_{n}", tag="ws")
                    nc.vector.tensor_add(ws[:, :], t36[:, 0:512], pin[:, 36:CW])
                    nc.vector.tensor_mul(wsa[:, o:o + 512], ws[:, :], abc[:, o:o + 512])

                if b > 0:
                    for _n in range(NT):
                        tree_chunk(_n)

                # ---- stage 1: g1 matmuls + epilogue -> h1 ----
                h1 = pb.tile([128, G, T + 4], BF16, name=f"h1_{b}", tag="h1")
                h2 = pb.tile([128, G, T + 4], BF16, name=f"h2_{b}", tag="h2", bufs=1)
                for m in range(G):
                    nc.vector.memset(h1[:, m, 0:2], 0.0)
                    nc.vector.memset(h1[:, m, T + 2:T + 4], 0.0)
                    nc.vector.memset(h2[:, m, 0:2], 0.0)
                    nc.vector.memset(h2[:, m, T + 2:T + 4], 0.0)

                def adawin_tile(m, n, fcw, fbt, fb_nz, xn_tile, h_out, uid,
                                fast_ramp=False):
                    """gamma/beta matmuls + (1+g)*xn + b epilogue + lrelu -> h_out chunk."""
                    ns = slice(n * 512, (n + 1) * 512)
                    pga = pg.tile([128, 512], F32, name=f"pga{uid}", tag="pg")
                    pgb = pg.tile([128, 512], F32, name=f"pgb{uid}", tag="pg")
                    if fb_nz:
                        nc.tensor.matmul(pga[:, :], fbt[:, m * 128:(m + 1) * 128],
                                         ca[:, ns], start=True, stop=False)
                        nc.tensor.matmul(pga[:, :], fcw[:, m * 128:(m + 1) * 128],
                                         wsa[:, ns], start=False, stop=True)
                        nc.tensor.matmul(pgb[:, :], fbt[:, C + m * 128:C + (m + 1) * 128],
                                         ca[:, ns], start=True, stop=False)
                        nc.tensor.matmul(pgb[:, :], fcw[:, C + m * 128:C + (m + 1) * 128],
                                         wsa[:, ns], start=False, stop=True)
                    else:
                        nc.tensor.matmul(pga[:, :], fcw[:, m * 128:(m + 1) * 128],
                                         wsa[:, ns], start=True, stop=True)
                        nc.tensor.matmul(pgb[:, :], fcw[:, C + m * 128:C + (m + 1) * 128],
                                         wsa[:, ns], start=True, stop=True)
                    if fast_ramp:
                        # ACT evacuates the gamma bank and folds the +1; keeps
                        # the ramp's serial DVE chain short (ACT is idle here).
                        cg = sm.tile([128, 512], BF16, name=f"cg{uid}", tag="u")
                        nc.scalar.activation(cg[:, :], pga[:, :], AF.Identity,
                                             bias=1.0)
                        u = sm.tile([128, 512], BF16, name=f"uf{uid}", tag="v")
                        nc.vector.tensor_mul(u[:, :], cg[:, :], xn_tile[:, :])
                        w = sm.tile([128, 512], BF16, name=f"w{uid}", tag="w")
                        nc.vector.tensor_add(w[:, :], u[:, :], pgb[:, :])
                    else:
                        u = sm.tile([128, 512], BF16, name=f"u{uid}", tag="u")
                        nc.vector.tensor_mul(u[:, :], pga[:, :], xn_tile[:, :])
                        v = sm.tile([128, 512], BF16, name=f"v{uid}", tag="v")
                        nc.vector.tensor_add(v[:, :], u[:, :], xn_tile[:, :])
                        w = sm.tile([128, 512], BF16, name=f"w{uid}", tag="w")
                        nc.vector.tensor_add(w[:, :], v[:, :], pgb[:, :])
                    nc.scalar.activation(h_out[:, m, 2 + n * 512:2 + (n + 1) * 512],
                                         w[:, :], AF.Prelu, alpha=SLOPE)

                def stage1_chunk(n):
                    for m in range(G):
                        xn = sm.tile([128, 512], BF16, name=f"xn1_{b}_{m}_{n}", tag="xn1")
                        nc.scalar.activation(xn, xs[:, m, n * 512:(n + 1) * 512],
                                             AF.Tanh, scale=alpha1 * SQRT2)
                        adawin_tile(m, n, fc1w, fb1 if fc1b_nz else None,
                                    fc1b_nz, xn, h1, f"1_{b}_{m}_{n}",
                                    fast_ramp=(n < 2))
                    if b == 0 and n == 0:
                        warmup("pc", 12, rhs=wsa[:, n * 512:(n + 1) * 512])

                if b == 0:
                    tree_chunk(0)
                    tree_chunk(1)
                    stage1_chunk(0)
                    stage1_chunk(1)
                    tree_chunk(2)
                    tree_chunk(3)
                    stage1_chunk(2)
                    stage1_chunk(3)
                else:
                    for n in range(NT):
                        stage1_chunk(n)

                # ---- conv1 -> tanh -> stage 2 -> h2 ----
                for n in range(NT):
                    for m in range(G):
                        pct = pc.tile([128, 512], F32, name=f"pc1_{b}_{m}_{n}", tag="pc")
                        for k in range(3):
                            for ki in range(G):
                                lhsT = c1w[:, k, ki, m * 128:(m + 1) * 128]
                                rhs = h1[:, ki, 1 + k + n * 512:1 + k + (n + 1) * 512]
                                nc.tensor.matmul(pct[:, :], lhsT, rhs,
                                                 start=(k == 0 and ki == 0),
                                                 stop=(k == 2 and ki == G - 1))
                        xn2 = sm.tile([128, 512], BF16, name=f"xn2_{b}_{m}_{n}", tag="xn2")
                        nc.scalar.activation(xn2, pct[:, :], AF.Tanh,
                                             bias=ab2[:, m:m + 1], scale=alpha2)
                        adawin_tile(m, n, fc2w, None if not fc2b_nz else fb2,
                                    fc2b_nz, xn2, h2, f"2_{b}_{m}_{n}")

                # ---- conv2 + residual -> out ----
                for n in range(NT):
                    for m in range(G):
                        pct = pc.tile([128, 512], F32, name=f"pc2_{b}_{m}_{n}", tag="pc")
                        for k in range(3):
                            for ki in range(G):
                                lhsT = c2w[:, k, ki, m * 128:(m + 1) * 128]
                                rhs = h2[:, ki, 1 + k + n * 512:1 + k + (n + 1) * 512]
                                nc.tensor.matmul(pct[:, :], lhsT, rhs,
                                                 start=(k == 0 and ki == 0),
                                                 stop=(k == 2 and ki == G - 1))
                        ot = sm.tile([128, 512], F32, name=f"ot{b}_{m}_{n}", tag="ot")
                        nc.vector.tensor_add(ot[:, :], pct[:, :],
                                             xs[:, m, n * 512:(n + 1) * 512])
                        if cb2_nz:
                            nc.scalar.activation(ot[:, :], ot[:, :], AF.Identity,
                                                 bias=cb2[:, m:m + 1])
                        dma(out=out_d[b, m, :, n * 512:(n + 1) * 512], in_=ot[:, :])
    return nc


_CACHE = {}


def _get_nc(fc1b_nz, fc2b_nz, cb2_nz, alpha1, alpha2):
    key = (fc1b_nz, fc2b_nz, cb2_nz, float(alpha1), float(alpha2))
    if key not in _CACHE:
        nc = bacc.Bacc("TRN2", target_bir_lowering=False, debug=False,
                       num_devices=NCORES)
        _build(nc, fc1b_nz, fc2b_nz, cb2_nz, alpha1, alpha2)
        nc.compile()
        _CACHE[key] = nc
    return _CACHE[key]


def _host_prep(x, s, lengths, fc1_w, fc1_b, alpha1, conv1_w, conv1_b,
               fc2_w, fc2_b, alpha2, conv2_w, conv2_b):
    """Host-side input preparation. Returns (in_maps, meta)."""
    x = np.asarray(x, np.float32)
    s = np.asarray(s).astype(ml_dtypes.bfloat16)
    lengths = np.asarray(lengths)
    a1 = float(np.asarray(alpha1).reshape(-1)[0])
    a2 = float(np.asarray(alpha2).reshape(-1)[0])

    # A = mask / (win_sum(mask) + eps), cnt = win_sum(ones)  -- all [B, T]
    t_idx = np.arange(T)
    mask = (t_idx[None, :] < lengths[:, None]).astype(np.float64)
    kern = np.ones(W_LEN)
    den = np.stack([np.convolve(mask[i], kern, mode="same") for i in range(B)]) + EPS
    A = (mask / den).astype(ml_dtypes.bfloat16)
    cnt = np.convolve(np.ones(T), kern, mode="same")
    cA = (A * cnt[None, :]).astype(ml_dtypes.bfloat16)

    bf = ml_dtypes.bfloat16
    xs = (x / SQRT2).reshape(B, G, 128, T).astype(bf)
    fc1wT = np.ascontiguousarray(fc1_w.T).astype(bf)             # [S, 2C]
    fc2wT = np.ascontiguousarray(fc2_w.T).astype(bf)
    # conv weights: [O, I, 3] -> [p=i%128, k, ki=i//128, o]
    c1wT = np.ascontiguousarray(
        conv1_w.astype(np.float32).transpose(1, 2, 0).reshape(G, 128, 3, C)
        .transpose(1, 2, 0, 3)).astype(bf)
    c2wT = np.ascontiguousarray(
        (conv2_w.astype(np.float32) / SQRT2).transpose(1, 2, 0).reshape(G, 128, 3, C)
        .transpose(1, 2, 0, 3)).astype(bf)
    ab2 = (a2 * conv1_b.astype(np.float32)).reshape(G, 128)
    cb2 = (conv2_b.astype(np.float32) / SQRT2).reshape(G, 128)
    fb1 = fc1_b.astype(bf).reshape(1, 2 * C)
    fb2 = fc2_b.astype(bf).reshape(1, 2 * C)
    fc1b_nz = bool(np.any(fc1_b))
    fc2b_nz = bool(np.any(fc2_b))
    cb2_nz = bool(np.any(conv2_b))

    in_maps = []
    for c in range(NCORES):
        bs = slice(c * BL, (c + 1) * BL)
        in_maps.append({
            "xs": xs[bs], "s": s[bs], "a": A[bs], "ca": cA[bs],
            "fc1w": fc1wT, "fc2w": fc2wT, "c1w": c1wT, "c2w": c2wT,
            "ab2": ab2, "cb2": cb2, "fb1": fb1, "fb2": fb2,
        })
    return in_maps, (fc1b_nz, fc2b_nz, cb2_nz, a1, a2)


def kernel(x, s, lengths, fc1_w, fc1_b, alpha1, conv1_w, conv1_b,
           fc2_w, fc2_b, alpha2, conv2_w, conv2_b, _trace=False):
    in_maps, (fc1b_nz, fc2b_nz, cb2_nz, a1, a2) = _host_prep(
        x, s, lengths, fc1_w, fc1_b, alpha1, conv1_w, conv1_b,
        fc2_w, fc2_b, alpha2, conv2_w, conv2_b)
    nc = _get_nc(fc1b_nz, fc2b_nz, cb2_nz, a1, a2)
    res = run_bass_kernel_spmd(nc, in_maps, core_ids=list(range(NCORES)),
                               trace=_trace)
    out = np.concatenate([res.results[i]["out"] for i in range(NCORES)], axis=0)
    out = out.reshape(B, C, T).astype(np.float32)
    kernel.last_exec_time_ns = res.exec_time_ns
    return out
